# revision 58
# baseline (speedup 1.0000x reference)
"""GQA attention layer (B=2, S=2048, H=4096, 32 Q heads / 8 KV heads, HD=128)
on 8 trn2 NeuronCores.

Sharding: 2D = data-parallel over batch (2) x tensor-parallel over heads (4).
Core c -> (batch = c // 4, tp = c % 4): 8 Q heads, 2 KV heads, full sequence.
Wq/Wk/Wv split along output rows, Wo along input cols (Megatron TP); the
4 per-batch partial outputs are summed on the host (the TP unshard step).

All matmuls run in bf16 (1 cycle/col).  x^T is streamed once into SBUF
during phase A and stays resident through phase B.  Roped Q lives in SBUF
for the whole kernel (dual-side tile-pool stacks give it a non-nested
lifetime), so phase C never touches DRAM for activations.  Per-core phases:
  A: K/V projections (streams x into SBUF), RoPE on K     -> ktr, vb (SBUF)
  B: Q projection + RoPE                                  -> qtr (SBUF)
  C: attention per (q-chunk, head-pair), see below        -> ctx (SBUF, bf16)
  D: out = ctx^T x Wo^T (bf16, fp32 accum)                -> out (DRAM, bf16)

Phase C processes heads in pairs sharing the kv head; one exp (ACT) per kt
covers both heads.  The softmax denominator: exp pair-sums on DVE (one wide
TT per kt-pair) feed ones-matmuls that accumulate per-group in PSUM.  Causal
q-chunks handle the 4 diagonal kt tiles at 128-column granularity: the
scores/exp/AV/denominator all run on columns [d*128, 512) only, and a single
shared 128x128 lower-triangle pattern multiplies the diagonal block after
exp.  All items run in one flat software pipeline (AV/den lag by 4 slots,
normalization pre/post by 5/7) so the PE never waits on ACT/DVE.

B's PSUM is split into even/odd-head pools released as soon as their last
rope is emitted, so C's score matmuls enter the PE queue immediately behind
B's last matmul (no inter-phase bubble).

RoPE runs in the natural interleaved head layout: pair (x[2i], x[2i+1])
sits at adjacent partitions, the partner is fetched with a swap-adjacent
stream_shuffle, and the sign/cos/sin tables are pre-interleaved on the host:
  rot = x * cc + shuffle(x * ss),  cc[2i]=cc[2i+1]=cos_i,
  ss[2i]=+sin_i, ss[2i+1]=-sin_i.
"""

import math

import numpy as np
import ml_dtypes

import concourse.bass as bass
import concourse.mybir as mybir
import concourse.tile as tile
from concourse import bacc
from concourse import bass_utils
from concourse.bass_interp import get_hw_module

B, S, H, NH, NKV, HD = 2, 2048, 4096, 32, 8, 128
TP = 4  # head-parallel cores per batch
N_CORES = 8
QH = NH // TP          # 8 q heads per core
KVH = NKV // TP        # 2 kv heads per core
QROWS = QH * HD        # 1024
KVROWS = KVH * HD      # 256
HT = H // 128          # 32 h (contraction) tiles
ST = S // 128          # 16 seq tiles
QCH = 512              # q-chunk width in phase C
NQQ = S // QCH
F32 = mybir.dt.float32
BF16 = mybir.dt.bfloat16
AX = mybir.AluOpType
ACTF = mybir.ActivationFunctionType
SWAP_ADJ = [i ^ 1 for i in range(32)]


def build_nc(mode: str, debug: bool = False):
    causal = mode == "causal"
    genmask = mode == "genmask"

    nc = bacc.Bacc("TRN2", target_bir_lowering=False, debug=False, num_devices=N_CORES)
    xtb = nc.dram_tensor("xtb", [H, S], BF16, kind="ExternalInput").ap()
    # host pre-packs weights so every SBUF weight tile is one contiguous DMA
    wqt = nc.dram_tensor("wqt", [QH, 128, HT, 128], BF16, kind="ExternalInput").ap()
    wkt = nc.dram_tensor("wkt", [128, HT, KVROWS], BF16, kind="ExternalInput").ap()
    wvt = nc.dram_tensor("wvt", [128, HT, KVROWS], BF16, kind="ExternalInput").ap()
    wot = nc.dram_tensor("wot", [QROWS, H], BF16, kind="ExternalInput").ap()
    cs = nc.dram_tensor("cs", [128, S], BF16, kind="ExternalInput").ap()
    sc = nc.dram_tensor("sc", [128, S], BF16, kind="ExternalInput").ap()
    tri = None
    maskt = None
    if causal:
        # tri[p, c] = 1.0 iff p <= c (shared diagonal-block mask)
        tri = nc.dram_tensor("tri", [128, 128], BF16, kind="ExternalInput").ap()
    if genmask:
        maskt = nc.dram_tensor("maskt", [S, S], BF16, kind="ExternalInput").ap()
    out = nc.dram_tensor("out", [S, H], BF16, kind="ExternalOutput").ap()
    dbg = {}
    if debug:
        dbg["k"] = nc.dram_tensor("dbg_k", [128, KVH * S], BF16, kind="ExternalOutput").ap()
        dbg["v"] = nc.dram_tensor("dbg_v", [128, KVH * ST * HD], BF16, kind="ExternalOutput").ap()
        dbg["q"] = nc.dram_tensor("dbg_q", [128, QH * S], BF16, kind="ExternalOutput").ap()
        dbg["ctx"] = nc.dram_tensor("dbg_ctx", [128, QH * S], BF16, kind="ExternalOutput").ap()

    with tile.TileContext(nc) as tc:
        pp = tc.alloc_tile_pool(name="persist", bufs=1, side="left")
        ktr = pp.tile([128, KVH, S], BF16)          # roped K^T (1 MB)
        vb = pp.tile([128, KVH, ST, HD], BF16)      # V [seq, hd] tiles (1 MB)
        ones_bf = pp.tile([128, 1], BF16)
        ones_row = pp.tile([1, 128], BF16)
        tri_sb = pp.tile([128, 128], BF16, name="tri_sb") if causal else None
        nc.gpsimd.memset(ones_bf[:], 1.0)
        nc.gpsimd.memset(ones_row[:], 1.0)
        # dummy exp pulls the ACT exp table-set load (~2.7us) into the
        # DMA-bound kernel start instead of phase C's first exp
        warm = pp.tile([1, 1], F32)
        nc.scalar.activation(warm[:], ones_row[:, 0:1], ACTF.Exp)

        # Right-side stack, bottom-up: cssc and wqp live to the end of the
        # kernel so their address zones are never reused (pool-zone reuse
        # inserts engine barriers; keeping them alive decouples phase C's
        # start from B's rope tail).
        cssc = tc.alloc_tile_pool(name="cssc", bufs=1, side="right")
        cs_sb = cssc.tile([128, S], BF16)
        sc_sb = cssc.tile([128, S], BF16)

        # wq head tiles (double-buffered; head 0 prefetched during A)
        wqp = tc.alloc_tile_pool(name="wq", bufs=2, side="right")
        wq_tiles = {}

        def load_wq(head):
            t = wqp.tile([128, HT, 128], BF16, name="wq_sb")
            nc.scalar.dma_start(t[:], wqt[head])
            wq_tiles[head] = t

        # ----- x resident through phases A+B (left) ---------------------
        xrp = tc.alloc_tile_pool(name="xres", bufs=1, side="left")
        x_sb = xrp.tile([128, HT, S], BF16)

        # ---------------- Phase A: K/V projection + K rope --------------
        wkvp = tc.alloc_tile_pool(name="wkv", bufs=1, side="right")
        wk_sb = wkvp.tile([128, HT, KVROWS], BF16)
        wv_sb = wkvp.tile([128, HT, KVROWS], BF16)
        with (
            tc.tile_pool(name="ropea", bufs=2) as rpa,
            tc.tile_pool(name="psk", bufs=2, space="PSUM") as psk,
            tc.tile_pool(name="psv", bufs=1, space="PSUM", side="right") as psv,
        ):
            # weight loads ride the ACT/DVE-adjacent rings so the sync ring
            # only carries x; chunked so the first K/V matmuls don't wait on
            # the whole transfer
            # wk/wv ride the fast sync ring, chunk-interleaved with the x
            # stream below so each chunk lands just ahead of its matmuls
            # (the scalar ring is ~3x slower and couldn't keep up with V).
            bounds = [0, 2, 8, 16, 24, 32]
            wk_issue = {0: 0, 1: 1, 8: 2, 16: 3, 24: 4}
            if causal:
                nc.gpsimd.dma_start(tri_sb[:], tri[:])
            for q4 in range(4):          # seq quarters of 512
                sl = slice(q4 * 512, (q4 + 1) * 512)
                kps = psk.tile([128, KVH, 512], F32, name="kps")
                vps = [psv.tile([128, KVROWS], F32, name=f"vps{st}")
                       for st in range(4)]
                for h in range(HT):
                    if q4 == 0 and h in wk_issue:
                        ci = wk_issue[h]
                        hsl = slice(bounds[ci], bounds[ci + 1])
                        nc.sync.dma_start(wk_sb[:, hsl, :], wkt[:, hsl, :])
                        nc.sync.dma_start(wv_sb[:, hsl, :], wvt[:, hsl, :])
                    xa = x_sb[:, h, sl]
                    nc.sync.dma_start(xa, xtb[h * 128:(h + 1) * 128, sl])
                    for r in range(KVH):
                        nc.tensor.matmul(kps[:, r, :],
                                         wk_sb[:, h, r * 128:(r + 1) * 128],
                                         xa,
                                         start=(h == 0), stop=(h == HT - 1))
                    for st in range(4):
                        nc.tensor.matmul(vps[st][:],
                                         xa[:, st * 128:(st + 1) * 128],
                                         wv_sb[:, h, :],
                                         start=(h == 0), stop=(h == HT - 1))
                if q4 == 0:
                    nc.scalar.dma_start(cs_sb[:], cs[:])
                    nc.scalar.dma_start(sc_sb[:], sc[:])
                    load_wq(0)
                # evict V first (split ScE/DVE) so the single-buffered vps
                # banks free before the next quarter's V matmuls arrive:
                # vps[st][p, kv*128+d] -> vb[p, kv, q4*4+st, d]
                for st in range(4):
                    src = vps[st][:].rearrange("p (kv d) -> p kv d", kv=KVH)
                    dst = vb[:, :, q4 * 4 + st, :]
                    if st < 2:
                        nc.scalar.copy(dst, src)
                    else:
                        nc.vector.tensor_copy(dst, src)
                # rope K -> ktr: rot = x*cc + shuffle(x*ss); ScE drains PSUM
                # to bf16 first so the DVE multiplies run in 2x mode
                for r in range(KVH):
                    kp = rpa.tile([128, 512], BF16, name="kp", bufs=2)
                    nc.scalar.copy(kp[:], kps[:, r, :])
                    # DVE-only temps: in-order engine, no double-buffer needed
                    t1 = rpa.tile([128, 512], BF16, name="t1", bufs=1)
                    m0 = rpa.tile([128, 512], BF16, name="m0", bufs=1)
                    sw = rpa.tile([128, 512], BF16, name="sw", bufs=1)
                    nc.vector.tensor_tensor(t1[:], kp[:], cs_sb[:, sl], op=AX.mult)
                    nc.vector.tensor_tensor(m0[:], kp[:], sc_sb[:, sl], op=AX.mult)
                    nc.vector.stream_shuffle(sw[:], m0[:], mask=SWAP_ADJ)
                    nc.vector.tensor_tensor(ktr[:, r, sl], t1[:], sw[:], op=AX.add)
        wkvp.release()

        # roped Q^T, SBUF-resident through phase C (right-side stack).
        # One tile per head-pair so phase C's first groups depend only on
        # their own heads' rope (tile-granular deps), not B's full tail.
        qtrp = tc.alloc_tile_pool(name="qtr", bufs=1, side="right")
        qtr_hp = [qtrp.tile([128, 2, S], BF16, name=f"qtr{i}")
                  for i in range(QH // 2)]

        # ---------------- Phase B: Q projection + rope ------------------
        # per-head stationary wq tile; for a fixed (head, h-tile) the
        # stationary weight feeds all 4 seq chunks back-to-back.
        # Even heads use the left PSUM pool, odd heads the right one, so
        # the even pool can release (for phase C) before B finishes.
        psbE = tc.alloc_tile_pool(name="psbE", bufs=1, space="PSUM", side="left")
        psbL = tc.alloc_tile_pool(name="psbL", bufs=1, space="PSUM", side="right")
        rpb_pool = tc.alloc_tile_pool(name="ropeb", bufs=2, side="right")
        for head in range(QH):
            if head + 1 < QH:
                load_wq(head + 1)
            wq_sb = wq_tiles.pop(head)
            qps = (psbE if head % 2 == 0 else psbL).tile(
                [128, 4, 512], F32, name="qps")
            for h in range(HT):
                for qc in range(4):
                    nc.tensor.matmul(
                        qps[:, qc, :],
                        wq_sb[:, h, :],
                        x_sb[:, h, qc * 512:(qc + 1) * 512],
                        start=(h == 0), stop=(h == HT - 1))
            for qc in range(4):
                sl = slice(qc * 512, (qc + 1) * 512)
                qp = rpb_pool.tile([128, 512], BF16, name="qp", bufs=2)
                nc.scalar.copy(qp[:], qps[:, qc, :])
                t1 = rpb_pool.tile([128, 512], BF16, name="t1", bufs=1)
                m0 = rpb_pool.tile([128, 512], BF16, name="m0", bufs=1)
                sw = rpb_pool.tile([128, 512], BF16, name="sw", bufs=1)
                nc.vector.tensor_tensor(t1[:], qp[:], cs_sb[:, sl], op=AX.mult)
                nc.vector.tensor_tensor(m0[:], qp[:], sc_sb[:, sl], op=AX.mult)
                nc.vector.stream_shuffle(sw[:], m0[:], mask=SWAP_ADJ)
                nc.vector.tensor_tensor(qtr_hp[head // 2][:, head % 2, sl],
                                        t1[:], sw[:], op=AX.add)
            if head == QH - 2:
                psbE.release()      # frees 4 banks for phase C's scores
        psbL.release()
        xrp.release()
        # rpb_pool / wqp / cssc stay allocated to the end: releasing them
        # here would chain phase C's pool allocs onto B's rope tail.

        if debug:
            for i in range(QH // 2):
                nc.sync.dma_start(
                    dbg["q"][:, i * 2 * S:(i + 1) * 2 * S],
                    qtr_hp[i][:].rearrange("p h s -> p (h s)"))

        # ---------------- Phases C+D scope ------------------------------
        # ctx/wo pool is allocated lazily a few pipeline slots into phase C:
        # its alloc boundary waits on B's rope tail (address reuse of x), and
        # emitting it (plus the wo-prefetch DMAs) before C's first matmuls
        # would stall them behind it in the sync queue.
        cd = {}

        def alloc_ctx():
            # right side: stacks above the live rope pools, so the only
            # released zone it can overlap is x's (freed at B's last matmul)
            ctxp = tc.alloc_tile_pool(name="ctxp", bufs=1, side="right")
            cd["ctxp"] = ctxp
            cd["ctx"] = ctxp.tile([128, QH, S], BF16, name="ctx")

        def alloc_wo():
            wop = tc.alloc_tile_pool(name="wop", bufs=1, side="right")
            cd["wop"] = wop
            wo_sb = wop.tile([128, QH, H], BF16, name="wo_sb")
            for h in range(QH):
                nc.sync.dma_start(wo_sb[:, h, :], wot[h * 128:(h + 1) * 128, :])
            cd["wo"] = wo_sb

        # ---------------- Phase C: attention ----------------------------
        # pscs reuses psbE's banks (released after head 6) so the first
        # scores run right behind B's last matmuls.  pscd/pscx reuse psbL's
        # banks (released after head 7's PSUM drain) — their pool allocs are
        # emitted a few slots into C so the alloc barrier doesn't stall the
        # exp/scores streams.
        pscs = tc.alloc_tile_pool(name="pscs", bufs=2, space="PSUM", side="left")
        cp = {}

        def alloc_cpsum():
            cp["pscd"] = tc.alloc_tile_pool(name="pscd", bufs=1, space="PSUM",
                                            side="left")
            cp["pscx"] = tc.alloc_tile_pool(name="pscx", bufs=2, space="PSUM",
                                            side="right")
        with (
            tc.tile_pool(name="expp", bufs=6 if genmask else 8) as expp,
            tc.tile_pool(name="sump", bufs=2 if genmask else 3) as sump,
            tc.tile_pool(name="smallc", bufs=2) as smc,
            tc.tile_pool(name="mkp", bufs=1) as mkp,
        ):
            # Flat item list: one item per (q-chunk, head-pair, kt).  For
            # causal chunks the 4 diagonal kt tiles come first (j == d) and
            # run at partial width [d*128, 512).
            items = []
            for qq in range(NQQ):
                NKT = 4 * qq + 4 if causal else ST
                if causal:
                    order = list(range(NKT - 4, NKT)) + list(range(NKT - 4))
                else:
                    order = list(range(NKT))
                for hp in range(QH // 2):
                    for j, kt in enumerate(order):
                        items.append((qq, hp, j, kt, NKT))

            # Slot s emission order: ITEM(s) scores first (unblocks the ACT
            # exp chain asap), then DEN(s-4)+RECIP, UNCOPY(s-5), AV(s-4),
            # POST(s-6).  RECIP right after the group's last dps MM and
            # UNCOPY before the next group's first AV keep the 1-buffered
            # dps / 2-slot ctxps PSUM reuse off the PE critical path.
            LAG_AV, LAG_DEN, LAG_PRE, LAG_POST = 4, 5, 4, 8

            mks = {}

            def load_mk(qq):
                mk = mkp.tile([128, ST, QCH], BF16, name="mk")
                for kt in range(ST):
                    nc.sync.dma_start(
                        mk[:, kt, :],
                        maskt[kt * 128:(kt + 1) * 128,
                              qq * QCH:(qq + 1) * QCH])
                return mk

            if genmask:
                mks[0] = load_mk(0)

            def isdiag(j):
                return causal and j < 4

            st8 = {}        # (qq, hp) -> group state
            pres = {}       # (qq, hp) -> normalize_pre result
            N = len(items)
            for s in range(N + LAG_POST + 1):
                if s == 3:
                    alloc_cpsum()
                if s == 5:
                    alloc_ctx()
                if s == 7 and not genmask:
                    # genmask has no SBUF room for wo during C; it loads wo
                    # between C and D instead (insurance path, slower)
                    alloc_wo()
                # ---- this slot's item: scores + exp (+mask, +pair-sum) --
                if s < N:
                    qq, hp, j, kt, NKT = items[s]
                    g = (qq, hp)
                    if j == 0:
                        if hp == 0 and genmask and qq not in mks:
                            mks[qq] = load_mk(qq)
                            mks.pop(qq - 1, None)
                        # ctxps/dps tiles are created lazily at first use
                        # (AV/DEN phases, >= 4 slots later) so their pools
                        # can be allocated after C's pipeline is rolling
                        st8[g] = dict(
                            eps={}, esums={},
                            heads=(2 * hp, 2 * hp + 1),
                            kvh=hp // (QH // KVH // 2),
                            qsl=slice(qq * QCH, (qq + 1) * QCH),
                            NKT=NKT, qq=qq,
                            j0=4 if causal else 0)
                    gs = st8[g]
                    cs_off = j * 128 if isdiag(j) else 0
                    sp = pscs.tile([128, 2, QCH], F32, name="sp")
                    for i in range(2):
                        nc.tensor.matmul(
                            sp[:, i, cs_off:],
                            ktr[:, gs["kvh"], kt * 128:(kt + 1) * 128],
                            qtr_hp[hp][:, i,
                                       qq * QCH + cs_off:(qq + 1) * QCH],
                            start=True, stop=True)
                        if genmask:
                            nc.vector.tensor_tensor(
                                sp[:, i, :], sp[:, i, :],
                                mks[qq][:, kt, :], op=AX.add)
                    ep = expp.tile([128, 2, QCH], BF16, name="ep")
                    nc.scalar.activation(ep[:, :, cs_off:], sp[:, :, cs_off:],
                                         ACTF.Exp)
                    if isdiag(j):
                        # multiplicative lower-triangle mask on the 128-wide
                        # diagonal block
                        blk = slice(cs_off, cs_off + 128)
                        for i in range(2):
                            nc.vector.tensor_tensor(
                                ep[:, i, blk], ep[:, i, blk], tri_sb[:],
                                op=AX.mult)
                    gs["eps"][j] = ep
                    jj = j - gs["j0"]
                    if not isdiag(j) and jj % 2 == 1:
                        # pair-sums alternate between GpSimd and DVE so
                        # neither backs up in the dense (qq=3) stretches
                        es = sump.tile([128, 2, QCH], BF16, name="es")
                        if (jj // 2) % 2 == 0:
                            for i in range(2):
                                nc.gpsimd.tensor_tensor(
                                    es[:, i, :], gs["eps"][j - 1][:, i, :],
                                    ep[:, i, :], op=AX.add)
                        else:
                            nc.vector.tensor_tensor(
                                es[:], gs["eps"][j - 1][:], ep[:], op=AX.add)
                        gs["esums"][j] = es
                # ---- denominator matmuls at s-LAG_DEN, + reciprocal -----
                sd = s - LAG_DEN
                if 0 <= sd < N:
                    qq, hp, j, kt, NKT = items[sd]
                    gs = st8[(qq, hp)]
                    if "dps" not in gs:
                        gs["dps"] = cp["pscd"].tile([1, 2, QCH], F32, name="dps")
                    if isdiag(j):
                        cs_off = j * 128
                        for i in range(2):
                            nc.tensor.matmul(
                                gs["dps"][0:1, i, cs_off:], ones_bf[:],
                                gs["eps"][j][:, i, cs_off:],
                                start=(j == 0),
                                stop=(j == NKT - 1))
                    elif (j - gs["j0"]) % 2 == 1:
                        es = gs["esums"][j]
                        for i in range(2):
                            nc.tensor.matmul(
                                gs["dps"][0:1, i, :], ones_bf[:], es[:, i, :],
                                start=(not causal and j == 1),
                                stop=(j == NKT - 1))
                    if j == NKT - 1:
                        # reciprocal right behind the group's last dps MM so
                        # the 1-buffered dps tile recycles without stalling
                        # the next group's first dps; bf16 out feeds the
                        # broadcast matmul directly (fp32 matmul = 2 passes)
                        rf = smc.tile([1, 2, QCH], F32, name="rf", bufs=1)
                        nc.vector.reciprocal_approx_fast(rf[:], gs["dps"][0:1, :, :])
                        rb = smc.tile([1, 2, QCH], BF16, name="rb")
                        nc.vector.tensor_copy(rb[:], rf[:])
                        pres[(qq, hp)][0] = rb
                # ---- AV matmuls: j<NKT-1 at s-4, group-last at s-3 ------
                def do_av(sa):
                    qq, hp, j, kt, NKT = items[sa]
                    gs = st8[(qq, hp)]
                    if j == 0:
                        gs["ctxps"] = [cp["pscx"].tile([128, QCH], F32,
                                                       name="ctxps")
                                       for _ in range(2)]
                    ep = gs["eps"][j]
                    cs_off = j * 128 if isdiag(j) else 0
                    for i in range(2):
                        nc.tensor.matmul(
                            gs["ctxps"][i][:, cs_off:], vb[:, gs["kvh"], kt, :],
                            ep[:, i, cs_off:],
                            start=(j == 0), stop=(j == NKT - 1))
                sa = s - LAG_AV
                if 0 <= sa < N and items[sa][2] != items[sa][4] - 1:
                    do_av(sa)
                sa2 = s - (LAG_AV - 1)
                if 0 <= sa2 < N and items[sa2][2] == items[sa2][4] - 1:
                    do_av(sa2)
                # ---- ctx drain (un-copy) at s-LAG_PRE, after the early
                # group-last AV so the next group's AV j=0 (one slot later)
                # finds the ctxps banks already drained
                spre = s - LAG_PRE
                if 0 <= spre < N:
                    qq, hp, j, kt, NKT = items[spre]
                    if j == NKT - 1:
                        gs = st8[(qq, hp)]
                        uns = []
                        for i in range(2):
                            un = smc.tile([128, QCH], BF16, name="un")
                            nc.vector.tensor_copy(un[:], gs["ctxps"][i][:])
                            uns.append(un)
                        pres[(qq, hp)] = [None, uns, gs["heads"], gs["qsl"]]
                # ---- normalize post (broadcast 1/den + multiply); the
                # final group posts at lag 6 (rb ready at +5) to shorten
                # the pipeline drain into phase D
                so = s - LAG_POST
                if so == N - 1:
                    so = -1          # last item's post handled at lag 6
                if s - 6 == N - 1:
                    so = N - 1
                if 0 <= so < N:
                    qq, hp, j, kt, NKT = items[so]
                    if j == NKT - 1:
                        rf, uns, heads, qsl = pres.pop((qq, hp))
                        bps = pscs.tile([128, 2, QCH], F32, name="sp")
                        for i in range(2):
                            nc.tensor.matmul(bps[:, i, :], ones_row[:],
                                             rf[0:1, i, :], start=True, stop=True)
                            nc.vector.tensor_tensor(
                                cd["ctx"][:, heads[i], qsl], uns[i], bps[:, i, :],
                                op=AX.mult)
                        st8.pop((qq, hp))

        cp["pscx"].release()
        cp["pscd"].release()
        pscs.release()
        if genmask:
            alloc_wo()
        ctx = cd["ctx"]
        wo_sb = cd["wo"]

        if debug:
            nc.sync.dma_start(dbg["k"][:], ktr[:].rearrange("p kv s -> p (kv s)"))
            nc.sync.dma_start(dbg["v"][:], vb[:].rearrange("p kv st d -> p (kv st d)"))
            nc.sync.dma_start(dbg["ctx"][:], ctx[:].rearrange("p h s -> p (h s)"))

        # ---------------- Phase D: output projection --------------------
        with (
            tc.tile_pool(name="ob", bufs=3) as obp,
            tc.tile_pool(name="psd", bufs=2, space="PSUM") as psd,
        ):
            for st in range(ST):
                for half in range(2):
                    ops = psd.tile([128, 2048], F32, name="ops")
                    for h in range(QH):
                        for n in range(4):
                            nc.tensor.matmul(
                                ops[:, n * 512:(n + 1) * 512],
                                ctx[:, h, st * 128:(st + 1) * 128],
                                wo_sb[:, h, half * 2048 + n * 512:half * 2048 + (n + 1) * 512],
                                start=(h == 0), stop=(h == QH - 1))
                    osb = obp.tile([128, 2048], BF16, name="osb")
                    # chunked eviction/writeback pipelines the tail
                    for n in range(4):
                        csl = slice(n * 512, (n + 1) * 512)
                        nc.scalar.copy(osb[:, csl], ops[:, csl])
                        nc.sync.dma_start(
                            out[st * 128:(st + 1) * 128,
                                half * 2048 + n * 512:half * 2048 + (n + 1) * 512],
                            osb[:, csl])
        cd["wop"].release()
        cd["ctxp"].release()
        rpb_pool.release()
        qtrp.release()
        wqp.release()
        cssc.release()
        pp.release()

    nc.compile()
    nc.m = get_hw_module(nc.m)
    return nc


_NC_CACHE = {}


def _get_nc(mode: str, debug: bool = False):
    key = (mode, debug)
    if key not in _NC_CACHE:
        _NC_CACHE[key] = build_nc(mode, debug)
    return _NC_CACHE[key]


def _detect_mode(attention_mask):
    if not np.any(attention_mask):
        return "nomask"
    tril = np.tril(np.ones((S, S), dtype=bool))
    for b in range(attention_mask.shape[0]):
        m = attention_mask[b, 0]
        if not (np.all(m[tril] == 0.0) and np.all(m[~tril] < -1e30)):
            return "genmask"
    return "causal"


def _build_tri():
    # tri[p, c] = 1.0 iff p <= c (valid below/on the diagonal of the block)
    k = np.arange(128)[:, None]
    c = np.arange(128)[None, :]
    return (k <= c).astype(ml_dtypes.bfloat16)


def kernel(hidden_states, cos, sin, position_ids, attention_mask, Wq, Wk, Wv, Wo,
           _trace=False, _debug=False):
    hidden_states = np.asarray(hidden_states, np.float32)
    cos = np.asarray(cos, np.float32)
    sin = np.asarray(sin, np.float32)
    position_ids = np.asarray(position_ids)
    attention_mask = np.asarray(attention_mask, np.float32)
    Wq = np.asarray(Wq, np.float32)
    Wk = np.asarray(Wk, np.float32)
    Wv = np.asarray(Wv, np.float32)
    Wo = np.asarray(Wo, np.float32)

    mode = _detect_mode(attention_mask)
    nc = _get_nc(mode, _debug)

    scale = 1.0 / math.sqrt(HD)
    wqt_full = np.ascontiguousarray((Wq * scale).T).astype(ml_dtypes.bfloat16)
    wkt_full = np.ascontiguousarray(Wk.T).astype(ml_dtypes.bfloat16)
    wvt_full = np.ascontiguousarray(Wv.T).astype(ml_dtypes.bfloat16)
    wot_full = np.ascontiguousarray(Wo.T).astype(ml_dtypes.bfloat16)

    pos = np.asarray(position_ids, np.int64)
    tri = _build_tri() if mode == "causal" else None
    per_batch = {}
    for b in range(B):
        xtb = np.ascontiguousarray(hidden_states[b].T).astype(ml_dtypes.bfloat16)
        cg = cos[pos[b]]                                     # [2048, 64]
        sg = sin[pos[b]]
        cs_b = np.repeat(cg.T, 2, axis=0).astype(ml_dtypes.bfloat16)   # cc
        sc_b = np.empty((HD, S), np.float32)                           # ss
        sc_b[0::2] = sg.T
        sc_b[1::2] = -sg.T
        sc_b = sc_b.astype(ml_dtypes.bfloat16)
        mt_b = None
        if mode == "genmask":
            mt_b = np.ascontiguousarray(attention_mask[b, 0].T).astype(ml_dtypes.bfloat16)
        per_batch[b] = (xtb, cs_b, sc_b, mt_b)

    in_maps = []
    for c in range(N_CORES):
        b, tp = c // TP, c % TP
        xtb, cs_b, sc_b, mt_b = per_batch[b]
        # wq slice -> [QH, 128(p), HT, 128(c)]: head-major contiguous tiles
        wq_sl = wqt_full[:, tp * QROWS:(tp + 1) * QROWS]     # [H, QROWS]
        wq_r = np.ascontiguousarray(
            wq_sl.reshape(HT, 128, QH, 128).transpose(2, 1, 0, 3))
        wk_sl = wkt_full[:, tp * KVROWS:(tp + 1) * KVROWS]   # [H, KVROWS]
        wk_r = np.ascontiguousarray(
            wk_sl.reshape(HT, 128, KVROWS).transpose(1, 0, 2))
        wv_sl = wvt_full[:, tp * KVROWS:(tp + 1) * KVROWS]
        wv_r = np.ascontiguousarray(
            wv_sl.reshape(HT, 128, KVROWS).transpose(1, 0, 2))
        m = {
            "xtb": xtb,
            "wqt": wq_r,
            "wkt": wk_r,
            "wvt": wv_r,
            "wot": np.ascontiguousarray(wot_full[tp * QROWS:(tp + 1) * QROWS, :]),
            "cs": cs_b,
            "sc": sc_b,
        }
        if mode == "causal":
            m["tri"] = tri
        if mode == "genmask":
            m["maskt"] = mt_b
        in_maps.append(m)

    res = bass_utils.run_bass_kernel_spmd(
        nc, in_maps, core_ids=list(range(N_CORES)), trace=_trace)

    out = np.zeros((B, S, H), np.float32)
    for c in range(N_CORES):
        out[c // TP] += np.asarray(res.results[c]["out"], np.float32)
    if _trace:
        kernel._last_results = res
    return out


# revision 59
# speedup vs baseline: 1.1865x; 1.1865x over previous
"""GQA attention layer (B=2, S=2048, H=4096, 32 Q heads / 8 KV heads, HD=128)
on 8 trn2 NeuronCores.

Sharding: 2D = data-parallel over batch (2) x tensor-parallel over heads (4).
Core c -> (batch = c // 4, tp = c % 4): 8 Q heads, 2 KV heads, full sequence.
Wq/Wk/Wv split along output rows, Wo along input cols (Megatron TP); the
4 per-batch partial outputs are summed on the host (the TP unshard step).

All matmuls run in bf16 (1 cycle/col).  x^T is streamed once into SBUF
during phase A and stays resident through phase B.  Roped Q lives in SBUF
for the whole kernel (dual-side tile-pool stacks give it a non-nested
lifetime), so phase C never touches DRAM for activations.  Per-core phases:
  A: K/V projections (streams x into SBUF), RoPE on K     -> ktr, vb (SBUF)
  B: Q projection + RoPE                                  -> qtr (SBUF)
  C: attention per (q-chunk, head-pair), see below        -> ctx (SBUF, bf16)
  D: out = ctx^T x Wo^T (bf16, fp32 accum)                -> out (DRAM, bf16)

Phase C processes heads in pairs sharing the kv head; one exp (ACT) per kt
covers both heads.  The softmax denominator: exp pair-sums on DVE (one wide
TT per kt-pair) feed ones-matmuls that accumulate per-group in PSUM.  Causal
q-chunks handle the 4 diagonal kt tiles at 128-column granularity: the
scores/exp/AV/denominator all run on columns [d*128, 512) only, and a single
shared 128x128 lower-triangle pattern multiplies the diagonal block after
exp.  All items run in one flat software pipeline (AV/den lag by 4 slots,
normalization pre/post by 5/7) so the PE never waits on ACT/DVE.

B's PSUM is split into even/odd-head pools released as soon as their last
rope is emitted, so C's score matmuls enter the PE queue immediately behind
B's last matmul (no inter-phase bubble).

RoPE runs in the natural interleaved head layout: pair (x[2i], x[2i+1])
sits at adjacent partitions, the partner is fetched with a swap-adjacent
stream_shuffle, and the sign/cos/sin tables are pre-interleaved on the host:
  rot = x * cc + shuffle(x * ss),  cc[2i]=cc[2i+1]=cos_i,
  ss[2i]=+sin_i, ss[2i+1]=-sin_i.
"""

import math

import numpy as np
import ml_dtypes

import concourse.bass as bass
import concourse.mybir as mybir
import concourse.tile as tile
from concourse import bacc
from concourse import bass_utils
from concourse.bass_interp import get_hw_module

B, S, H, NH, NKV, HD = 2, 2048, 4096, 32, 8, 128
TP = 4  # head-parallel cores per batch
N_CORES = 8
QH = NH // TP          # 8 q heads per core
KVH = NKV // TP        # 2 kv heads per core
QROWS = QH * HD        # 1024
KVROWS = KVH * HD      # 256
HT = H // 128          # 32 h (contraction) tiles
ST = S // 128          # 16 seq tiles
QCH = 512              # q-chunk width in phase C
NQQ = S // QCH
F32 = mybir.dt.float32
BF16 = mybir.dt.bfloat16
AX = mybir.AluOpType
ACTF = mybir.ActivationFunctionType
SWAP_ADJ = [i ^ 1 for i in range(32)]


def build_nc(mode: str, debug: bool = False):
    causal = mode == "causal"
    genmask = mode == "genmask"

    nc = bacc.Bacc("TRN2", target_bir_lowering=False, debug=False, num_devices=N_CORES)
    xtb = nc.dram_tensor("xtb", [H, S], BF16, kind="ExternalInput").ap()
    # host pre-packs weights so every SBUF weight tile is one contiguous DMA
    wqt = nc.dram_tensor("wqt", [QH, 128, HT, 128], BF16, kind="ExternalInput").ap()
    wkt = nc.dram_tensor("wkt", [128, HT, KVROWS], BF16, kind="ExternalInput").ap()
    wvt = nc.dram_tensor("wvt", [128, HT, KVROWS], BF16, kind="ExternalInput").ap()
    wot = nc.dram_tensor("wot", [QROWS, H], BF16, kind="ExternalInput").ap()
    cs = nc.dram_tensor("cs", [128, S], BF16, kind="ExternalInput").ap()
    sc = nc.dram_tensor("sc", [128, S], BF16, kind="ExternalInput").ap()
    tri = None
    maskt = None
    if causal:
        # tri[p, c] = 1.0 iff p <= c (shared diagonal-block mask)
        tri = nc.dram_tensor("tri", [128, 128], BF16, kind="ExternalInput").ap()
    if genmask:
        maskt = nc.dram_tensor("maskt", [S, S], BF16, kind="ExternalInput").ap()
    out = nc.dram_tensor("out", [S, H], BF16, kind="ExternalOutput").ap()
    dbg = {}
    if debug:
        dbg["k"] = nc.dram_tensor("dbg_k", [128, KVH * S], BF16, kind="ExternalOutput").ap()
        dbg["v"] = nc.dram_tensor("dbg_v", [128, KVH * ST * HD], BF16, kind="ExternalOutput").ap()
        dbg["q"] = nc.dram_tensor("dbg_q", [128, QH * S], BF16, kind="ExternalOutput").ap()
        dbg["ctx"] = nc.dram_tensor("dbg_ctx", [128, QH * S], BF16, kind="ExternalOutput").ap()

    with tile.TileContext(nc) as tc:
        pp = tc.alloc_tile_pool(name="persist", bufs=1, side="left")
        ktr = pp.tile([128, KVH, S], BF16)          # roped K^T (1 MB)
        vb = pp.tile([128, KVH, ST, HD], BF16)      # V [seq, hd] tiles (1 MB)
        ones_bf = pp.tile([128, 1], BF16)
        ones_row = pp.tile([1, 128], BF16)
        tri_sb = pp.tile([128, 128], BF16, name="tri_sb") if causal else None
        nc.gpsimd.memset(ones_bf[:], 1.0)
        nc.gpsimd.memset(ones_row[:], 1.0)
        # dummy exp pulls the ACT exp table-set load (~2.7us) into the
        # DMA-bound kernel start instead of phase C's first exp
        warm = pp.tile([1, 1], F32)
        nc.scalar.activation(warm[:], ones_row[:, 0:1], ACTF.Exp)

        # Right-side stack, bottom-up: cssc and wqp live to the end of the
        # kernel so their address zones are never reused (pool-zone reuse
        # inserts engine barriers; keeping them alive decouples phase C's
        # start from B's rope tail).
        cssc = tc.alloc_tile_pool(name="cssc", bufs=1, side="right")
        cs_sb = cssc.tile([128, S], BF16)
        sc_sb = cssc.tile([128, S], BF16)

        # wq head tiles (double-buffered; head 0 prefetched during A)
        wqp = tc.alloc_tile_pool(name="wq", bufs=2, side="right")
        wq_tiles = {}

        def load_wq(head):
            t = wqp.tile([128, HT, 128], BF16, name="wq_sb")
            nc.scalar.dma_start(t[:], wqt[head])
            wq_tiles[head] = t

        # ----- x resident through phases A+B (left) ---------------------
        xrp = tc.alloc_tile_pool(name="xres", bufs=1, side="left")
        x_sb = xrp.tile([128, HT, S], BF16)

        # ---------------- Phase A: K/V projection + K rope --------------
        wkvp = tc.alloc_tile_pool(name="wkv", bufs=1, side="right")
        wk_sb = wkvp.tile([128, HT, KVROWS], BF16)
        wv_sb = wkvp.tile([128, HT, KVROWS], BF16)
        with (
            tc.tile_pool(name="ropea", bufs=2) as rpa,
            tc.tile_pool(name="psk", bufs=2, space="PSUM") as psk,
            tc.tile_pool(name="psv", bufs=1, space="PSUM", side="right") as psv,
        ):
            # weight loads ride the ACT/DVE-adjacent rings so the sync ring
            # only carries x; chunked so the first K/V matmuls don't wait on
            # the whole transfer
            # wk/wv ride the fast sync ring, chunk-interleaved with the x
            # stream below so each chunk lands just ahead of its matmuls
            # (the scalar ring is ~3x slower and couldn't keep up with V).
            bounds = [0, 2, 8, 16, 24, 32]
            wk_issue = {0: 0, 1: 1, 8: 2, 16: 3, 24: 4}
            if causal:
                nc.gpsimd.dma_start(tri_sb[:], tri[:])
            for q4 in range(4):          # seq quarters of 512
                sl = slice(q4 * 512, (q4 + 1) * 512)
                kps = psk.tile([128, KVH, 512], F32, name="kps")
                vps = [psv.tile([128, KVROWS], F32, name=f"vps{st}")
                       for st in range(4)]
                for h in range(HT):
                    if q4 == 0 and h in wk_issue:
                        ci = wk_issue[h]
                        hsl = slice(bounds[ci], bounds[ci + 1])
                        nc.sync.dma_start(wk_sb[:, hsl, :], wkt[:, hsl, :])
                    xa = x_sb[:, h, sl]
                    nc.sync.dma_start(xa, xtb[h * 128:(h + 1) * 128, sl])
                    if q4 == 0 and h in wk_issue:
                        # wv after x: V matmuls trail K's, so the first
                        # K matmul's critical path is wk_c0 + x_h0 only
                        nc.sync.dma_start(wv_sb[:, hsl, :], wvt[:, hsl, :])
                    for r in range(KVH):
                        nc.tensor.matmul(kps[:, r, :],
                                         wk_sb[:, h, r * 128:(r + 1) * 128],
                                         xa,
                                         start=(h == 0), stop=(h == HT - 1))
                    for st in range(4):
                        nc.tensor.matmul(vps[st][:],
                                         xa[:, st * 128:(st + 1) * 128],
                                         wv_sb[:, h, :],
                                         start=(h == 0), stop=(h == HT - 1))
                if q4 == 0:
                    nc.scalar.dma_start(cs_sb[:], cs[:])
                    nc.scalar.dma_start(sc_sb[:], sc[:])
                    load_wq(0)
                # evict V first (split ScE/DVE) so the single-buffered vps
                # banks free before the next quarter's V matmuls arrive:
                # vps[st][p, kv*128+d] -> vb[p, kv, q4*4+st, d]
                for st in range(4):
                    src = vps[st][:].rearrange("p (kv d) -> p kv d", kv=KVH)
                    dst = vb[:, :, q4 * 4 + st, :]
                    if st < 2:
                        nc.scalar.copy(dst, src)
                    else:
                        nc.vector.tensor_copy(dst, src)
                # rope K -> ktr: rot = x*cc + shuffle(x*ss); ScE drains PSUM
                # to bf16 first so the DVE multiplies run in 2x mode
                for r in range(KVH):
                    kp = rpa.tile([128, 512], BF16, name="kp", bufs=2)
                    nc.scalar.copy(kp[:], kps[:, r, :])
                    # DVE-only temps: in-order engine, no double-buffer needed
                    t1 = rpa.tile([128, 512], BF16, name="t1", bufs=1)
                    m0 = rpa.tile([128, 512], BF16, name="m0", bufs=1)
                    sw = rpa.tile([128, 512], BF16, name="sw", bufs=1)
                    nc.vector.tensor_tensor(t1[:], kp[:], cs_sb[:, sl], op=AX.mult)
                    nc.vector.tensor_tensor(m0[:], kp[:], sc_sb[:, sl], op=AX.mult)
                    nc.vector.stream_shuffle(sw[:], m0[:], mask=SWAP_ADJ)
                    nc.vector.tensor_tensor(ktr[:, r, sl], t1[:], sw[:], op=AX.add)
        wkvp.release()

        # roped Q^T, SBUF-resident through phase C (right-side stack).
        # One tile per head-pair so phase C's first groups depend only on
        # their own heads' rope (tile-granular deps), not B's full tail.
        qtrp = tc.alloc_tile_pool(name="qtr", bufs=1, side="right")
        qtr_hp = [qtrp.tile([128, 2, S], BF16, name=f"qtr{i}")
                  for i in range(QH // 2)]

        # ---------------- Phase B: Q projection + rope ------------------
        # per-head stationary wq tile; for a fixed (head, h-tile) the
        # stationary weight feeds all 4 seq chunks back-to-back.
        # Even heads use the left PSUM pool, odd heads the right one, so
        # the even pool can release (for phase C) before B finishes.
        psbE = tc.alloc_tile_pool(name="psbE", bufs=1, space="PSUM", side="left")
        psbL = tc.alloc_tile_pool(name="psbL", bufs=1, space="PSUM", side="right")
        rpb_pool = tc.alloc_tile_pool(name="ropeb", bufs=2, side="right")
        for head in range(QH):
            if head + 1 < QH:
                load_wq(head + 1)
            wq_sb = wq_tiles.pop(head)
            qps = (psbE if head % 2 == 0 else psbL).tile(
                [128, 4, 512], F32, name="qps")
            for h in range(HT):
                for qc in range(4):
                    nc.tensor.matmul(
                        qps[:, qc, :],
                        wq_sb[:, h, :],
                        x_sb[:, h, qc * 512:(qc + 1) * 512],
                        start=(h == 0), stop=(h == HT - 1))
            for qc in range(4):
                sl = slice(qc * 512, (qc + 1) * 512)
                qp = rpb_pool.tile([128, 512], BF16, name="qp", bufs=2)
                nc.scalar.copy(qp[:], qps[:, qc, :])
                t1 = rpb_pool.tile([128, 512], BF16, name="t1", bufs=1)
                m0 = rpb_pool.tile([128, 512], BF16, name="m0", bufs=1)
                sw = rpb_pool.tile([128, 512], BF16, name="sw", bufs=1)
                nc.vector.tensor_tensor(t1[:], qp[:], cs_sb[:, sl], op=AX.mult)
                nc.vector.tensor_tensor(m0[:], qp[:], sc_sb[:, sl], op=AX.mult)
                nc.vector.stream_shuffle(sw[:], m0[:], mask=SWAP_ADJ)
                nc.vector.tensor_tensor(qtr_hp[head // 2][:, head % 2, sl],
                                        t1[:], sw[:], op=AX.add)
            if head == QH - 2:
                psbE.release()      # frees 4 banks for phase C's scores
        psbL.release()
        xrp.release()
        # rpb_pool / wqp / cssc stay allocated to the end: releasing them
        # here would chain phase C's pool allocs onto B's rope tail.

        if debug:
            for i in range(QH // 2):
                nc.sync.dma_start(
                    dbg["q"][:, i * 2 * S:(i + 1) * 2 * S],
                    qtr_hp[i][:].rearrange("p h s -> p (h s)"))

        # ---------------- Phases C+D scope ------------------------------
        # ctx/wo pool is allocated lazily a few pipeline slots into phase C:
        # its alloc boundary waits on B's rope tail (address reuse of x), and
        # emitting it (plus the wo-prefetch DMAs) before C's first matmuls
        # would stall them behind it in the sync queue.
        cd = {}

        def alloc_ctx():
            # right side: stacks above the live rope pools, so the only
            # released zone it can overlap is x's (freed at B's last matmul)
            ctxp = tc.alloc_tile_pool(name="ctxp", bufs=1, side="right")
            cd["ctxp"] = ctxp
            cd["ctx"] = ctxp.tile([128, QH, S], BF16, name="ctx")

        def alloc_wo():
            wop = tc.alloc_tile_pool(name="wop", bufs=1, side="right")
            cd["wop"] = wop
            wo_sb = wop.tile([128, QH, H], BF16, name="wo_sb")
            for h in range(QH):
                nc.sync.dma_start(wo_sb[:, h, :], wot[h * 128:(h + 1) * 128, :])
            cd["wo"] = wo_sb

        # ---------------- Phase C: attention ----------------------------
        # pscs reuses psbE's banks (released after head 6) so the first
        # scores run right behind B's last matmuls.  pscd/pscx reuse psbL's
        # banks (released after head 7's PSUM drain) — their pool allocs are
        # emitted a few slots into C so the alloc barrier doesn't stall the
        # exp/scores streams.
        pscs = tc.alloc_tile_pool(name="pscs", bufs=2, space="PSUM", side="left")
        cp = {}

        def alloc_cpsum():
            cp["pscd"] = tc.alloc_tile_pool(name="pscd", bufs=1, space="PSUM",
                                            side="left")
            cp["pscx"] = tc.alloc_tile_pool(name="pscx", bufs=2, space="PSUM",
                                            side="right")
        with (
            tc.tile_pool(name="expp", bufs=6 if genmask else 8) as expp,
            tc.tile_pool(name="sump", bufs=2 if genmask else 3) as sump,
            tc.tile_pool(name="smallc", bufs=2) as smc,
            tc.tile_pool(name="mkp", bufs=1) as mkp,
        ):
            # Flat item list: one item per (q-chunk, head-pair, kt).  For
            # causal chunks the 4 diagonal kt tiles come first (j == d) and
            # run at partial width [d*128, 512).
            items = []
            for qq in range(NQQ):
                NKT = 4 * qq + 4 if causal else ST
                if causal:
                    order = list(range(NKT - 4, NKT)) + list(range(NKT - 4))
                else:
                    order = list(range(NKT))
                for hp in range(QH // 2):
                    for j, kt in enumerate(order):
                        items.append((qq, hp, j, kt, NKT))

            # Slot s emission order: ITEM(s) scores first (unblocks the ACT
            # exp chain asap), then DEN(s-4)+RECIP, UNCOPY(s-5), AV(s-4),
            # POST(s-6).  RECIP right after the group's last dps MM and
            # UNCOPY before the next group's first AV keep the 1-buffered
            # dps / 2-slot ctxps PSUM reuse off the PE critical path.
            LAG_AV, LAG_DEN, LAG_PRE, LAG_POST = 4, 5, 4, 8

            mks = {}

            def load_mk(qq):
                mk = mkp.tile([128, ST, QCH], BF16, name="mk")
                for kt in range(ST):
                    nc.sync.dma_start(
                        mk[:, kt, :],
                        maskt[kt * 128:(kt + 1) * 128,
                              qq * QCH:(qq + 1) * QCH])
                return mk

            if genmask:
                mks[0] = load_mk(0)

            def isdiag(j):
                return causal and j < 4

            st8 = {}        # (qq, hp) -> group state
            pres = {}       # (qq, hp) -> normalize_pre result
            N = len(items)
            for s in range(N + LAG_POST + 1):
                if s == 3:
                    alloc_cpsum()
                if s == 5:
                    alloc_ctx()
                if s == 7 and not genmask:
                    # genmask has no SBUF room for wo during C; it loads wo
                    # between C and D instead (insurance path, slower)
                    alloc_wo()
                # ---- this slot's item: scores + exp (+mask, +pair-sum) --
                if s < N:
                    qq, hp, j, kt, NKT = items[s]
                    g = (qq, hp)
                    if j == 0:
                        if hp == 0 and genmask and qq not in mks:
                            mks[qq] = load_mk(qq)
                            mks.pop(qq - 1, None)
                        # ctxps/dps tiles are created lazily at first use
                        # (AV/DEN phases, >= 4 slots later) so their pools
                        # can be allocated after C's pipeline is rolling
                        st8[g] = dict(
                            eps={}, esums={},
                            heads=(2 * hp, 2 * hp + 1),
                            kvh=hp // (QH // KVH // 2),
                            qsl=slice(qq * QCH, (qq + 1) * QCH),
                            NKT=NKT, qq=qq,
                            j0=4 if causal else 0)
                    gs = st8[g]
                    cs_off = j * 128 if isdiag(j) else 0
                    sp = pscs.tile([128, 2, QCH], F32, name="sp")
                    for i in range(2):
                        nc.tensor.matmul(
                            sp[:, i, cs_off:],
                            ktr[:, gs["kvh"], kt * 128:(kt + 1) * 128],
                            qtr_hp[hp][:, i,
                                       qq * QCH + cs_off:(qq + 1) * QCH],
                            start=True, stop=True)
                        if genmask:
                            nc.vector.tensor_tensor(
                                sp[:, i, :], sp[:, i, :],
                                mks[qq][:, kt, :], op=AX.add)
                    ep = expp.tile([128, 2, QCH], BF16, name="ep")
                    nc.scalar.activation(ep[:, :, cs_off:], sp[:, :, cs_off:],
                                         ACTF.Exp)
                    if isdiag(j):
                        # multiplicative lower-triangle mask on the 128-wide
                        # diagonal block
                        blk = slice(cs_off, cs_off + 128)
                        for i in range(2):
                            nc.vector.tensor_tensor(
                                ep[:, i, blk], ep[:, i, blk], tri_sb[:],
                                op=AX.mult)
                    gs["eps"][j] = ep
                    jj = j - gs["j0"]
                    if not isdiag(j) and jj % 2 == 1:
                        # pair-sums alternate between GpSimd and DVE so
                        # neither backs up in the dense (qq=3) stretches
                        es = sump.tile([128, 2, QCH], BF16, name="es")
                        if (jj // 2) % 2 == 0:
                            for i in range(2):
                                nc.gpsimd.tensor_tensor(
                                    es[:, i, :], gs["eps"][j - 1][:, i, :],
                                    ep[:, i, :], op=AX.add)
                        else:
                            nc.vector.tensor_tensor(
                                es[:], gs["eps"][j - 1][:], ep[:], op=AX.add)
                        gs["esums"][j] = es
                # ---- denominator matmuls at s-LAG_DEN, + reciprocal -----
                sd = s - LAG_DEN
                if 0 <= sd < N:
                    qq, hp, j, kt, NKT = items[sd]
                    gs = st8[(qq, hp)]
                    if "dps" not in gs:
                        gs["dps"] = cp["pscd"].tile([1, 2, QCH], F32, name="dps")
                    if isdiag(j):
                        cs_off = j * 128
                        for i in range(2):
                            nc.tensor.matmul(
                                gs["dps"][0:1, i, cs_off:], ones_bf[:],
                                gs["eps"][j][:, i, cs_off:],
                                start=(j == 0),
                                stop=(j == NKT - 1))
                    elif (j - gs["j0"]) % 2 == 1:
                        es = gs["esums"][j]
                        for i in range(2):
                            nc.tensor.matmul(
                                gs["dps"][0:1, i, :], ones_bf[:], es[:, i, :],
                                start=(not causal and j == 1),
                                stop=(j == NKT - 1))
                    if j == NKT - 1:
                        # reciprocal right behind the group's last dps MM so
                        # the 1-buffered dps tile recycles without stalling
                        # the next group's first dps; bf16 out feeds the
                        # broadcast matmul directly (fp32 matmul = 2 passes)
                        rf = smc.tile([1, 2, QCH], F32, name="rf", bufs=1)
                        nc.vector.reciprocal_approx_fast(rf[:], gs["dps"][0:1, :, :])
                        rb = smc.tile([1, 2, QCH], BF16, name="rb")
                        nc.vector.tensor_copy(rb[:], rf[:])
                        pres[(qq, hp)][0] = rb
                # ---- AV matmuls: j<NKT-1 at s-4, group-last at s-3 ------
                def do_av(sa):
                    qq, hp, j, kt, NKT = items[sa]
                    gs = st8[(qq, hp)]
                    if j == 0:
                        gs["ctxps"] = [cp["pscx"].tile([128, QCH], F32,
                                                       name="ctxps")
                                       for _ in range(2)]
                    ep = gs["eps"][j]
                    cs_off = j * 128 if isdiag(j) else 0
                    for i in range(2):
                        nc.tensor.matmul(
                            gs["ctxps"][i][:, cs_off:], vb[:, gs["kvh"], kt, :],
                            ep[:, i, cs_off:],
                            start=(j == 0), stop=(j == NKT - 1))
                sa = s - LAG_AV
                if 0 <= sa < N and items[sa][2] != items[sa][4] - 1:
                    do_av(sa)
                sa2 = s - (LAG_AV - 1)
                if 0 <= sa2 < N and items[sa2][2] == items[sa2][4] - 1:
                    do_av(sa2)
                # ---- ctx drain (un-copy) at s-LAG_PRE, after the early
                # group-last AV so the next group's AV j=0 (one slot later)
                # finds the ctxps banks already drained
                spre = s - LAG_PRE
                if 0 <= spre < N:
                    qq, hp, j, kt, NKT = items[spre]
                    if j == NKT - 1:
                        gs = st8[(qq, hp)]
                        uns = []
                        for i in range(2):
                            un = smc.tile([128, QCH], BF16, name="un")
                            nc.vector.tensor_copy(un[:], gs["ctxps"][i][:])
                            uns.append(un)
                        pres[(qq, hp)] = [None, uns, gs["heads"], gs["qsl"]]
                # ---- normalize post (broadcast 1/den + multiply); the
                # final group posts at lag 6 (rb ready at +5) to shorten
                # the pipeline drain into phase D
                so = s - LAG_POST
                if so == N - 1:
                    so = -1          # last item's post handled at lag 6
                if s - 6 == N - 1:
                    so = N - 1
                if 0 <= so < N:
                    qq, hp, j, kt, NKT = items[so]
                    if j == NKT - 1:
                        rf, uns, heads, qsl = pres.pop((qq, hp))
                        bps = pscs.tile([128, 2, QCH], F32, name="sp")
                        for i in range(2):
                            nc.tensor.matmul(bps[:, i, :], ones_row[:],
                                             rf[0:1, i, :], start=True, stop=True)
                            nc.vector.tensor_tensor(
                                cd["ctx"][:, heads[i], qsl], uns[i], bps[:, i, :],
                                op=AX.mult)
                        st8.pop((qq, hp))

        cp["pscx"].release()
        cp["pscd"].release()
        pscs.release()
        if genmask:
            alloc_wo()
        ctx = cd["ctx"]
        wo_sb = cd["wo"]

        if debug:
            nc.sync.dma_start(dbg["k"][:], ktr[:].rearrange("p kv s -> p (kv s)"))
            nc.sync.dma_start(dbg["v"][:], vb[:].rearrange("p kv st d -> p (kv st d)"))
            nc.sync.dma_start(dbg["ctx"][:], ctx[:].rearrange("p h s -> p (h s)"))

        # ---------------- Phase D: output projection --------------------
        with (
            tc.tile_pool(name="ob", bufs=3) as obp,
            tc.tile_pool(name="psd", bufs=2, space="PSUM") as psd,
        ):
            for st in range(ST):
                for half in range(2):
                    ops = psd.tile([128, 2048], F32, name="ops")
                    for h in range(QH):
                        for n in range(4):
                            nc.tensor.matmul(
                                ops[:, n * 512:(n + 1) * 512],
                                ctx[:, h, st * 128:(st + 1) * 128],
                                wo_sb[:, h, half * 2048 + n * 512:half * 2048 + (n + 1) * 512],
                                start=(h == 0), stop=(h == QH - 1))
                    osb = obp.tile([128, 2048], BF16, name="osb")
                    # chunked eviction/writeback pipelines the tail
                    for n in range(4):
                        csl = slice(n * 512, (n + 1) * 512)
                        nc.scalar.copy(osb[:, csl], ops[:, csl])
                        nc.sync.dma_start(
                            out[st * 128:(st + 1) * 128,
                                half * 2048 + n * 512:half * 2048 + (n + 1) * 512],
                            osb[:, csl])
        cd["wop"].release()
        cd["ctxp"].release()
        rpb_pool.release()
        qtrp.release()
        wqp.release()
        cssc.release()
        pp.release()

    nc.compile()
    nc.m = get_hw_module(nc.m)
    return nc


_NC_CACHE = {}


def _get_nc(mode: str, debug: bool = False):
    key = (mode, debug)
    if key not in _NC_CACHE:
        _NC_CACHE[key] = build_nc(mode, debug)
    return _NC_CACHE[key]


def _detect_mode(attention_mask):
    if not np.any(attention_mask):
        return "nomask"
    tril = np.tril(np.ones((S, S), dtype=bool))
    for b in range(attention_mask.shape[0]):
        m = attention_mask[b, 0]
        if not (np.all(m[tril] == 0.0) and np.all(m[~tril] < -1e30)):
            return "genmask"
    return "causal"


def _build_tri():
    # tri[p, c] = 1.0 iff p <= c (valid below/on the diagonal of the block)
    k = np.arange(128)[:, None]
    c = np.arange(128)[None, :]
    return (k <= c).astype(ml_dtypes.bfloat16)


def kernel(hidden_states, cos, sin, position_ids, attention_mask, Wq, Wk, Wv, Wo,
           _trace=False, _debug=False):
    hidden_states = np.asarray(hidden_states, np.float32)
    cos = np.asarray(cos, np.float32)
    sin = np.asarray(sin, np.float32)
    position_ids = np.asarray(position_ids)
    attention_mask = np.asarray(attention_mask, np.float32)
    Wq = np.asarray(Wq, np.float32)
    Wk = np.asarray(Wk, np.float32)
    Wv = np.asarray(Wv, np.float32)
    Wo = np.asarray(Wo, np.float32)

    mode = _detect_mode(attention_mask)
    nc = _get_nc(mode, _debug)

    scale = 1.0 / math.sqrt(HD)
    wqt_full = np.ascontiguousarray((Wq * scale).T).astype(ml_dtypes.bfloat16)
    wkt_full = np.ascontiguousarray(Wk.T).astype(ml_dtypes.bfloat16)
    wvt_full = np.ascontiguousarray(Wv.T).astype(ml_dtypes.bfloat16)
    wot_full = np.ascontiguousarray(Wo.T).astype(ml_dtypes.bfloat16)

    pos = np.asarray(position_ids, np.int64)
    tri = _build_tri() if mode == "causal" else None
    per_batch = {}
    for b in range(B):
        xtb = np.ascontiguousarray(hidden_states[b].T).astype(ml_dtypes.bfloat16)
        cg = cos[pos[b]]                                     # [2048, 64]
        sg = sin[pos[b]]
        cs_b = np.repeat(cg.T, 2, axis=0).astype(ml_dtypes.bfloat16)   # cc
        sc_b = np.empty((HD, S), np.float32)                           # ss
        sc_b[0::2] = sg.T
        sc_b[1::2] = -sg.T
        sc_b = sc_b.astype(ml_dtypes.bfloat16)
        mt_b = None
        if mode == "genmask":
            mt_b = np.ascontiguousarray(attention_mask[b, 0].T).astype(ml_dtypes.bfloat16)
        per_batch[b] = (xtb, cs_b, sc_b, mt_b)

    in_maps = []
    for c in range(N_CORES):
        b, tp = c // TP, c % TP
        xtb, cs_b, sc_b, mt_b = per_batch[b]
        # wq slice -> [QH, 128(p), HT, 128(c)]: head-major contiguous tiles
        wq_sl = wqt_full[:, tp * QROWS:(tp + 1) * QROWS]     # [H, QROWS]
        wq_r = np.ascontiguousarray(
            wq_sl.reshape(HT, 128, QH, 128).transpose(2, 1, 0, 3))
        wk_sl = wkt_full[:, tp * KVROWS:(tp + 1) * KVROWS]   # [H, KVROWS]
        wk_r = np.ascontiguousarray(
            wk_sl.reshape(HT, 128, KVROWS).transpose(1, 0, 2))
        wv_sl = wvt_full[:, tp * KVROWS:(tp + 1) * KVROWS]
        wv_r = np.ascontiguousarray(
            wv_sl.reshape(HT, 128, KVROWS).transpose(1, 0, 2))
        m = {
            "xtb": xtb,
            "wqt": wq_r,
            "wkt": wk_r,
            "wvt": wv_r,
            "wot": np.ascontiguousarray(wot_full[tp * QROWS:(tp + 1) * QROWS, :]),
            "cs": cs_b,
            "sc": sc_b,
        }
        if mode == "causal":
            m["tri"] = tri
        if mode == "genmask":
            m["maskt"] = mt_b
        in_maps.append(m)

    res = bass_utils.run_bass_kernel_spmd(
        nc, in_maps, core_ids=list(range(N_CORES)), trace=_trace)

    out = np.zeros((B, S, H), np.float32)
    for c in range(N_CORES):
        out[c // TP] += np.asarray(res.results[c]["out"], np.float32)
    if _trace:
        kernel._last_results = res
    return out
